# revision 35
# baseline (speedup 1.0000x reference)
"""Trainium2 Bass kernel for nn_DisentangleEncoder (B=64, L=200, D=256, K=8).

Data-parallel over batch: 8 sequences per NeuronCore x 8 cores.

Math (per branch, per sequence b, x = item_emb[b]):
  mu1/var1   = stats of x over D            (shared by score-LN and final-LN)
  ln1n       = (x - mu1) * rstd1,  rstd1 = 1/sqrt(var1+eps)
  lng5       = ln1n * g5                    (used by final stage AND score matmul)
  score      = softmax_K(lng5 @ M2T + c2),  M2T[d,k] = (g1/g5)[d]*ln2[k,d],
                                            c2[k] = b1 . ln2[k], ln2 = LN(intentions)
  xpf        = x + pos_fai
  xpfn       = (xpf - mu4) * rstd4          (stats of xpf over D)
  khT        = transpose(xpfn) * g4 + b4    (key_hat, D-on-partitions layout)
  keyvT      = khT + relu(W_wT.T @ khT + W_b)
  q          = LN(gather(xpf, seq_len-1) + rou) * g3 + b3   (gather via onehot matmul)
  w          = softmax_L(q . keyvT * SCALE)
  c[k,l]     = score[l,k] * w[l]
  alpha[k,l] = c * sqrt(var1+eps) / sqrt(c^2*var1 + eps)
Final: out = alphaL*lng5L + alphaG*lng5G + 2*b5

v2: bulk tensors in bf16 (inputs pre-converted on host, output returned as
bf16 and upcast on host — rel tolerance is 2e-2, bf16 keeps us ~1e-3).
Input loads batched 4-per-group; final combine staged per (b,lt) into a
[128, K*D] tile and shipped with ONE dma each (16 output DMAs total) to
keep the SP sequencer (~0.9us per DMA issue) off the critical path.

All sqrt/rsqrt go through exp(+-0.5*ln(v)) so the ACT engine needs only the
natural_log_exp_and_others table set -- a single ACT_TABLE_LOAD instead of
~69 switches (~2.7us each).
Softmaxes skip max-subtraction: |scaled logits| < ~20, safely inside fp32 exp.
"""

import numpy as np
import ml_dtypes
from contextlib import ExitStack

import concourse.bacc as bacc
import concourse.bass as bass
import concourse.tile as tile
from concourse import mybir
from concourse.bass_utils import run_bass_kernel_spmd

# Route all Exp/Ln activations to the single `natural_log_exp_and_others`
# table set: hide Exp/Ln from every other set so Bacc's table-load insertion
# never alternates sets (each switch costs ~2.7us on HW).
_orig_gat = bacc.get_activation_tables


def _gat_joint_exp_ln(arch):
    tabs = _orig_gat(arch)
    out = {}
    for name, fns in tabs.items():
        fns = set(fns)
        if name != "natural_log_exp_and_others":
            fns.discard(mybir.ActivationFunctionType.Exp)
            fns.discard(mybir.ActivationFunctionType.Ln)
        out[name] = fns
    return out


bacc.get_activation_tables = _gat_joint_exp_ln

B, L, D, K = 64, 200, 256, 8
NCORES = 8
BPC = B // NCORES          # sequences per core
EPS = 1e-5
SCALE = 1.0 / float(np.sqrt(D))
F32 = mybir.dt.float32
BF16 = mybir.dt.bfloat16
NPBF = ml_dtypes.bfloat16
LT = (128, 72)             # L split into two partition tiles
NS = 16                    # streams per core = BPC seqs x 2 branches
AX = mybir.AxisListType.X
OP = mybir.AluOpType
AF = mybir.ActivationFunctionType


def _bc(ap, p):
    """Broadcast a DRAM AP across p partitions (partition-step 0)."""
    return bass.AP(tensor=ap.tensor, offset=ap.offset, ap=[[0, p]] + list(ap.ap))


def _emit_consts(nc, tc, ctx, t):
    cp = ctx.enter_context(tc.tile_pool(name="consts", bufs=1))
    c = {}
    c["pos"] = []
    for lt in range(2):
        pt = cp.tile([128, D], BF16, name=f"c_pos{lt}")
        if LT[lt] < 128:
            nc.gpsimd.memset(pt[64:, :], 0.0)
        nc.sync.dma_start(out=pt[: LT[lt], :], in_=t["pos"][lt * 128 : lt * 128 + LT[lt], :])
        c["pos"].append(pt)
    for nm, w in (("g5bc", "g5"), ("b5x2bc", "b5x2")):
        bcst = cp.tile([128, D], BF16, name=f"c_{nm}")
        nc.sync.dma_start(out=bcst, in_=_bc(t[w][0, :], 128))
        c[nm] = bcst
    c2bc = cp.tile([128, K], F32, name="c_c2bc")
    nc.sync.dma_start(out=c2bc, in_=_bc(t["c2"][0, :], 128))
    c["c2bc"] = c2bc
    c["m2t"] = []
    for dh in range(2):
        mt = cp.tile([128, K], BF16, name=f"c_m2t{dh}")
        nc.sync.dma_start(out=mt, in_=t["m2t"][dh * 128 : (dh + 1) * 128, :])
        c["m2t"].append(mt)
    c["wwt"] = [[None, None], [None, None]]
    for di in range(2):
        for do in range(2):
            wt = cp.tile([128, 128], BF16, name=f"c_wwt{di}{do}")
            nc.sync.dma_start(
                out=wt, in_=t["wwt"][di * 128 : (di + 1) * 128, do * 128 : (do + 1) * 128]
            )
            c["wwt"][di][do] = wt
    for nm in ("g4", "b4", "wb"):
        c[nm] = []
        for dh in range(2):
            col = cp.tile([128, 1], F32, name=f"c_{nm}{dh}")
            nc.sync.dma_start(out=col, in_=t[nm][dh * 128 : (dh + 1) * 128, :])
            c[nm].append(col)
    for nm in ("rouc", "g3g4", "b3g4", "g3c", "b3c"):
        c[nm] = []
        for dh in range(2):
            col = cp.tile([128, 1], F32, name=f"c_{nm}{dh}")
            nc.sync.dma_start(out=col, in_=t[nm][dh * 128 : (dh + 1) * 128, :])
            c[nm].append(col)
    onesc = cp.tile([128, 1], F32, name="c_onesc")
    nc.gpsimd.memset(onesc, 1.0)
    c["onesc"] = onesc
    c["iop1"] = []
    for lt in range(2):
        col = cp.tile([128, 1], F32, name=f"c_iop1{lt}")
        nc.sync.dma_start(out=col, in_=t["iop1"][lt * 128 : (lt + 1) * 128, :])
        c["iop1"].append(col)
    # [NS,256] broadcast rows for the batched q chain
    for nm in ("g3", "b3", "rou", "g4r"):
        row = cp.tile([NS, D], F32, name=f"c_{nm}")
        nc.sync.dma_start(out=row, in_=_bc(t[nm][0, :], NS))
        c[nm] = row
    eye = cp.tile([128, 128], BF16, name="c_eye")
    nc.sync.dma_start(out=eye, in_=t["eye"][:, :])
    c["eye"] = eye
    eye32 = cp.tile([NS, NS], F32, name="c_eye32")
    nc.sync.dma_start(out=eye32, in_=t["eye32"][:, :])
    c["eye32"] = eye32
    epsc = cp.tile([128, 1], F32, name="c_epsc")
    nc.gpsimd.memset(epsc, EPS)
    c["epsc"] = epsc
    ones1 = cp.tile([1, 128], F32, name="c_ones1")
    nc.gpsimd.memset(ones1, 1.0)
    c["ones1"] = ones1
    return c


def _emit_body(nc, tc, c, pools, t, out_t, fmode=('AAAAATTT', 'DDDDDDDD'), amode='pool', kmix=2, vmode='dve', groups=2, s6b=4, lmode='dve', dbg=None):
    for g in range(groups):
        seqs = list(range(g * (BPC // groups), (g + 1) * (BPC // groups)))
        _emit_group(nc, tc, c, pools, t, out_t, fmode, amode, kmix, vmode, g, seqs, s6b, lmode, dbg)


def _emit_group(nc, tc, c, pools, t, out_t, fmode, amode, kmix, vmode, g, seqs, s6b=4, lmode='dve', dbg=None):
    """Batched-phase emission: all (seq,branch) streams of this group
    processed in wide ops to minimize instruction count."""
    blk, tmp, ps, psR, psq, op = pools
    xsrc = {0: t["xL"], 1: t["xG"]}
    nb = len(seqs)       # seqs in this group
    ns = 2 * nb          # streams in this group

    # ---- persistent per-iteration block tiles ----
    xa = blk.tile([128, 2, ns, D], BF16, name=f"xa_{g}", tag=f"xa_{g}")        # x (-> ln1n in place)
    pa = blk.tile([128, 2, ns, D], BF16, name=f"pa_{g}", tag=f"pa_{g}")        # xpf (-> xpfn in place)
    lga = blk.tile([128, 2, ns, D], BF16, name=f"lga_{g}", tag=f"lga_{g}")     # lng5
    # T-layout tiles: [d(128), st, dh, l-cols] — l-col c<128 is lt0 row c,
    # col 128+c is lt1 row 128+c (so cols 0:200 are l=0:200; 200:256 garbage)
    lgTn = blk.tile([128, ns, 2, 256], BF16, name=f"lgTn_{g}", tag=f"lgTn_{g}")
    khTn = blk.tile([128, ns, 2, 256], BF16, name=f"khTn_{g}", tag=f"khTn_{g}")
    rla = blk.tile([128, ns, 2, 200], BF16, name=f"rla_{g}", tag=f"rla_{g}")   # relu(W'@xpfnT+wb')
    mv1 = blk.tile([128, 2, ns, 2], F32, name=f"mv1_{g}", tag=f"mv1_{g}")
    mv4 = blk.tile([128, 2, ns, 2], F32, name=f"mv4_{g}", tag=f"mv4_{g}")
    aall = blk.tile([128, 2, ns, K], F32, name=f"aall_{g}", tag=f"aall_{g}")
    wT = [blk.tile([128, ns], F32, name=f"wT{lt}_{g}", tag=f"wT{lt}_{g}") for lt in range(2)]
    qT = blk.tile([128, 2 * ns], BF16, name=f"qT_{g}", tag=f"qT_{g}")
    qTg = blk.tile([128, 2 * ns], BF16, name=f"qTg_{g}", tag=f"qTg_{g}")
    psQ = psq.tile([128, 21 * ns], F32, name=f"psQ_{g}", tag=f"psQ_{g}")

    # ---- phase 1: onehots, batched loads, stats ----
    slrow = tmp.tile([1, nb], F32, name=f"slrow_{g}", tag=f"slrow_{g}", bufs=1)
    nc.sync.dma_start(out=slrow, in_=t["slf"][seqs[0] : seqs[0] + nb, 0])
    pbc = ps.tile([128, nb], F32, name=f"pbc_{g}", tag="pT")
    nc.tensor.matmul(pbc, c["ones1"], slrow, start=True, stop=True)
    oh = []
    for lt in range(2):
        o = tmp.tile([128, nb], BF16, name=f"oha{lt}_{g}", tag=f"oha{lt}_{g}", bufs=1)
        iop_b = bass.AP(tensor=c["iop1"][lt].tensor, offset=c["iop1"][lt].offset,
                        ap=[list(c["iop1"][lt].ap[0]), [0, nb]])
        nc.vector.tensor_tensor(out=o, in0=iop_b, in1=pbc, op=OP.is_equal)
        oh.append(o)
    # zero garbage tails once so batched ops stay finite
    nc.gpsimd.memset(xa[64:, 1, :, :], 0.0)
    nc.gpsimd.memset(aall[64:, 1, :, :], 0.0)
    nc.gpsimd.memset(wT[1][64:, :], 0.0)
    # batched loads: one DMA per (branch, lt) covering all seqs of the group
    for br in range(2):
        for lt in range(2):
            src = xsrc[br][seqs[0] : seqs[0] + nb, lt * 128 : lt * 128 + LT[lt], :]
            nc.sync.dma_start(
                out=xa[: LT[lt], lt, br * nb : (br + 1) * nb, :],
                in_=src.rearrange("b l d -> l b d"),
            )
    for st in range(ns):
        for lt in range(2):
            s6 = tmp.tile([128, 6], F32, name=f"s6_{g}_{st}{lt}", tag="s6", bufs=s6b)
            nc.vector.bn_stats(out=s6, in_=xa[:, lt, st, :])
            nc.vector.bn_aggr(out=mv1[:, lt, st, :], in_=s6)
    for st in range(ns):
        for lt in range(2):
            eng_a = nc.gpsimd if amode == 'pool' else nc.vector
            eng_a.tensor_add(pa[:, lt, st, :], xa[:, lt, st, :], c["pos"][lt])
            s6 = tmp.tile([128, 6], F32, name=f"s64_{g}_{st}{lt}", tag="s6", bufs=s6b)
            nc.vector.bn_stats(out=s6, in_=pa[:, lt, st, :])
            nc.vector.bn_aggr(out=mv4[:, lt, st, :], in_=s6)

    # ---- phase 2: transposed q-row gather into PSUM columns ----
    # All psQ matmuls are single-window (start&stop): a start=True clears
    # has_written for the whole 2KB zero region (bank-wide), so concurrent
    # multi-instruction accumulation windows in one bank are illegal.
    # Column map (xns): [0:2) gather lt0 (dh-major), [2:4) gather lt1,
    # [4:12) logits (4 terms x 2ns each, (lt,st)-minor), [12:14) mu/rs bcast,
    # [14:15) recip bcast, [15:17) sum_q (dh), [17:19) sum_q2 (dh),
    # [19:21) wsum (lt).
    for st in range(ns):
        b = st % nb
        for dh in range(2):
            for lt in range(2):
                nc.tensor.matmul(
                    psQ[:, (2 * lt + dh) * ns + st : (2 * lt + dh) * ns + st + 1],
                    pa[:, lt, st, dh * 128 : (dh + 1) * 128],
                    oh[lt][:, b : b + 1],
                    start=True, stop=True)

    # ---- phase 3: batched rstd/sqv chains (3+2 ACT ops for all streams) ----
    ln1e = blk.tile([128, 2, ns, 1], F32, name=f"ln1e_{g}", tag=f"ln1e_{g}")
    rs1 = blk.tile([128, 2, ns, 1], F32, name=f"rs1_{g}", tag=f"rs1_{g}")
    sqv1 = blk.tile([128, 2, ns, 1], F32, name=f"sqv1_{g}", tag=f"sqv1_{g}")
    rs4 = blk.tile([128, 2, ns, 1], F32, name=f"rs4_{g}", tag=f"rs4_{g}")
    nc.scalar.activation(out=ln1e, in_=mv1[:, :, :, 1:2], func=AF.Ln, bias=c["epsc"], scale=1.0)
    nc.scalar.activation(out=rs1, in_=ln1e, func=AF.Exp, bias=0.0, scale=-0.5)
    nc.scalar.activation(out=sqv1, in_=ln1e, func=AF.Exp, bias=0.0, scale=0.5)
    ln4e = tmp.tile([128, 2, ns, 1], F32, name=f"ln4e_{g}", tag=f"ln4e_{g}", bufs=1)
    nc.scalar.activation(out=ln4e, in_=mv4[:, :, :, 1:2], func=AF.Ln, bias=c["epsc"], scale=1.0)
    nc.scalar.activation(out=rs4, in_=ln4e, func=AF.Exp, bias=0.0, scale=-0.5)

    # ---- phase 4: normalized forms (in place) + lng5 ----
    for st in range(ns):
        for lt in range(2):
            nc.vector.tensor_scalar(
                xa[:, lt, st, :], xa[:, lt, st, :],
                mv1[:, lt, st, 0:1], rs1[:, lt, st, 0:1], OP.subtract, OP.mult,
            )
            eng_l = nc.gpsimd if lmode == 'pool' else nc.vector
            eng_l.tensor_mul(lga[:, lt, st, :], xa[:, lt, st, :], c["g5bc"])
            nc.vector.tensor_scalar(
                pa[:, lt, st, :], pa[:, lt, st, :],
                mv4[:, lt, st, 0:1], rs4[:, lt, st, 0:1], OP.subtract, OP.mult,
            )

    # ---- phase 5: DMA-engine transposes (lng5 -> lgTn, xpfn -> khTn) ----
    # g4/b4 affine of key_hat is NOT applied: it is folded into the W matmul
    # weights/bias (host) and into the q vector (softmax shift-invariance
    # drops the q.b4 constant).  One XBAR-transpose DMA per (tensor, lt);
    # lg on the SP queue, kh on the ACT hwdge queue.
    # NOTE: all XBAR transposes go through ONE queue: issuing them
    # concurrently from two queues corrupts both outputs on HW (single
    # XBAR unit; CoreSim does not model the conflict).
    tq = nc.scalar if vmode == 'act' else nc.sync
    if vmode == 'split':
        h = ns // 2
        for lt in range(2):
            for sh in range(2):
                sl = slice(sh * h, (sh + 1) * h)
                tq.dma_start(out=lgTn[:, sl, :, lt * 128 : (lt + 1) * 128],
                             in_=lga[:, lt, sl, :], transpose=True)
                tq.dma_start(out=khTn[:, sl, :, lt * 128 : (lt + 1) * 128],
                             in_=pa[:, lt, sl, :], transpose=True)
    else:
        for lt in range(2):
            tq.dma_start(out=lgTn[:, :, :, lt * 128 : (lt + 1) * 128],
                         in_=lga[:, lt, :, :], transpose=True)
            tq.dma_start(out=khTn[:, :, :, lt * 128 : (lt + 1) * 128],
                         in_=pa[:, lt, :, :], transpose=True)

    # ---- phase 6: R = relu(W'@xpfnT + wb') with W' = (W_w*g4).T (host) ----
    for st in range(ns):
        pRs = []
        for do in range(2):
            pR = psR.tile([128, 200], F32, name=f"pR{do}_{g}_{st}", tag=f"pR{do}")
            for di in range(2):
                nc.tensor.matmul(
                    pR, c["wwt"][di][do], khTn[:, st, di, 0:200], start=(di == 0), stop=(di == 1)
                )
            pRs.append(pR)
        for do in range(2):
            nc.scalar.activation(out=rla[:, st, do, :], in_=pRs[do],
                                 func=AF.Relu, bias=c["wb"][do], scale=1.0)

    # ---- phase 7: q LN in T layout (stats via PE row-reductions) ----
    def _as3(ap):   # flat [128, 2ns] -> [128, 2, ns] view (same memory)
        return bass.AP(tensor=ap.tensor, offset=ap.offset,
                       ap=[list(ap.ap[0]), [ns, 2], [1, ns]])
    def _dup2(ap):  # [128, ns] -> [128, 2, ns] broadcast over middle dim
        return bass.AP(tensor=ap.tensor, offset=ap.offset,
                       ap=[list(ap.ap[0]), [0, 2], list(ap.ap[-1])])
    qts = blk.tile([128, 2 * ns], F32, name=f"qts_{g}", tag=f"qts_{g}")
    nc.scalar.activation(out=qts, in_=psQ[:, 0 : 2 * ns], func=AF.Copy, bias=0.0, scale=1.0)
    nc.vector.tensor_tensor(out=qts, in0=qts, in1=psQ[:, 2 * ns : 4 * ns], op=OP.add)
    for dh in range(2):
        nc.vector.tensor_scalar_add(qts[:, dh * ns : (dh + 1) * ns],
                                    qts[:, dh * ns : (dh + 1) * ns], c["rouc"][dh])
    qsq = tmp.tile([128, 2 * ns], F32, name=f"qsq_{g}", tag=f"qsq_{g}", bufs=1)
    nc.scalar.square(qsq, qts)
    for dh in range(2):
        nc.tensor.matmul(psQ[0:1, (15 + dh) * ns : (16 + dh) * ns], c["onesc"],
                         qts[:, dh * ns : (dh + 1) * ns], start=True, stop=True)
        nc.tensor.matmul(psQ[0:1, (17 + dh) * ns : (18 + dh) * ns], c["onesc"],
                         qsq[:, dh * ns : (dh + 1) * ns], start=True, stop=True)
    murs = tmp.tile([1, 2 * ns], F32, name=f"murs_{g}", tag=f"murs_{g}", bufs=1)
    nc.vector.tensor_scalar_mul(murs[:, 0:ns], psQ[0:1, 15 * ns : 16 * ns], 1.0 / D)
    nc.vector.scalar_tensor_tensor(out=murs[:, 0:ns], in0=psQ[0:1, 16 * ns : 17 * ns],
                                   scalar=1.0 / D, in1=murs[:, 0:ns], op0=OP.mult, op1=OP.add)
    qv = tmp.tile([1, ns], F32, name=f"qv_{g}", tag=f"qv_{g}", bufs=1)
    nc.vector.tensor_scalar_mul(qv, psQ[0:1, 17 * ns : 18 * ns], 1.0 / D)
    nc.vector.scalar_tensor_tensor(out=qv, in0=psQ[0:1, 18 * ns : 19 * ns],
                                   scalar=1.0 / D, in1=qv, op0=OP.mult, op1=OP.add)
    mm2 = tmp.tile([1, ns], F32, name=f"mm2_{g}", tag=f"mm2_{g}", bufs=1)
    nc.vector.tensor_tensor(out=mm2, in0=murs[:, 0:ns], in1=murs[:, 0:ns], op=OP.mult)
    nc.vector.tensor_tensor(out=qv, in0=qv, in1=mm2, op=OP.subtract)
    nc.scalar.activation(out=qv, in_=qv, func=AF.Ln, bias=c["epsc"][0:1, :], scale=1.0)
    nc.scalar.activation(out=murs[:, ns : 2 * ns], in_=qv, func=AF.Exp, bias=0.0, scale=-0.5)
    nc.tensor.matmul(psQ[:, 12 * ns : 14 * ns], c["ones1"], murs, start=True, stop=True)
    nc.vector.tensor_tensor(out=_as3(qts[:, :]), in0=_as3(qts[:, :]),
                            in1=_dup2(psQ[:, 12 * ns : 13 * ns]), op=OP.subtract)
    nc.vector.tensor_tensor(out=_as3(qts[:, :]), in0=_as3(qts[:, :]),
                            in1=_dup2(psQ[:, 13 * ns : 14 * ns]), op=OP.mult)
    for dh in range(2):
        nc.vector.tensor_scalar(qTg[:, dh * ns : (dh + 1) * ns], qts[:, dh * ns : (dh + 1) * ns],
                                c["g3g4"][dh], c["b3g4"][dh], OP.mult, OP.add)
        nc.vector.tensor_scalar(qT[:, dh * ns : (dh + 1) * ns], qts[:, dh * ns : (dh + 1) * ns],
                                c["g3c"][dh], c["b3c"][dh], OP.mult, OP.add)

    # ---- phase 8: w logitsT (khTn.q' + rla.q_raw) + softmax via PE sums ----
    for st in range(ns):
        for lt in range(2):
            n = LT[lt]
            for dh in range(2):
                nc.tensor.matmul(psQ[:n, (4 + 2 * dh) * ns + lt * ns + st : (4 + 2 * dh) * ns + lt * ns + st + 1],
                                 khTn[:, st, dh, lt * 128 : lt * 128 + n],
                                 qTg[:, dh * ns + st : dh * ns + st + 1], start=True, stop=True)
            for do in range(2):
                nc.tensor.matmul(psQ[:n, (8 + 2 * do) * ns + lt * ns + st : (8 + 2 * do) * ns + lt * ns + st + 1],
                                 rla[:, st, do, lt * 128 : lt * 128 + n],
                                 qT[:, do * ns + st : do * ns + st + 1], start=True, stop=True)
    logS = blk.tile([128, 2 * ns], F32, name=f"logS_{g}", tag=f"logS_{g}")
    for lt in range(2):
        n = LT[lt]
        d0 = logS[:n, lt * ns : (lt + 1) * ns]
        nc.scalar.activation(out=d0, in_=psQ[:n, (4 + lt) * ns : (5 + lt) * ns],
                             func=AF.Copy, bias=0.0, scale=1.0)
        nc.vector.tensor_tensor(out=d0, in0=d0,
                                in1=psQ[:n, (6 + lt) * ns : (7 + lt) * ns], op=OP.add)
        nc.vector.tensor_tensor(out=d0, in0=d0,
                                in1=psQ[:n, (8 + lt) * ns : (9 + lt) * ns], op=OP.add)
        nc.vector.tensor_tensor(out=d0, in0=d0,
                                in1=psQ[:n, (10 + lt) * ns : (11 + lt) * ns], op=OP.add)
    wex = [blk.tile([128, ns], F32, name=f"wex{lt}_{g}", tag=f"wex{lt}_{g}") for lt in range(2)]
    nc.gpsimd.memset(wex[1][64:, :], 0.0)
    for lt in range(2):
        n = LT[lt]
        nc.scalar.activation(out=wex[lt][:n, :], in_=logS[:n, lt * ns : (lt + 1) * ns],
                             func=AF.Exp, bias=0.0, scale=SCALE)
        nc.tensor.matmul(psQ[0:1, (19 + lt) * ns : (20 + lt) * ns], c["onesc"], wex[lt],
                         start=True, stop=True)
    wsum = tmp.tile([1, ns], F32, name=f"wsum_{g}", tag=f"wsum_{g}", bufs=1)
    nc.vector.tensor_copy(wsum, psQ[0:1, 19 * ns : 20 * ns])
    nc.vector.tensor_tensor(out=wsum, in0=wsum,
                            in1=psQ[0:1, 20 * ns : 21 * ns], op=OP.add)
    wrc = tmp.tile([1, ns], F32, name=f"wrc_{g}", tag=f"wrc_{g}", bufs=1)
    nc.vector.reciprocal(wrc, wsum)
    nc.tensor.matmul(psQ[:, 14 * ns : 15 * ns], c["ones1"], wrc, start=True, stop=True)
    for lt in range(2):
        nc.vector.tensor_tensor(out=wT[lt], in0=wex[lt], in1=psQ[:, 14 * ns : 15 * ns], op=OP.mult)

    # ---- phase 9: score matmuls + batched softmax/alpha ----
    for st in range(ns):
        for lt in range(2):
            n = LT[lt]
            pA = ps.tile([128, K], F32, name=f"pA{g}_{st}{lt}", tag="pT")
            for dh in range(2):
                nc.tensor.matmul(
                    pA[:n, :], lgTn[:, st, dh, lt * 128 : lt * 128 + n], c["m2t"][dh],
                    start=(dh == 0), stop=(dh == 1),
                )
            nc.vector.tensor_add(aall[:n, lt, st, :], pA[:n, :], c["c2bc"][:n, :])
    sex = blk.tile([128, 2, ns, K], F32, name=f"sex_{g}", tag=f"sex_{g}")
    nc.scalar.activation(out=sex, in_=aall, func=AF.Exp, bias=0.0, scale=SCALE)
    ssm = tmp.tile([128, 2, ns, 1], F32, name=f"ssm_{g}", tag=f"ssm_{g}", bufs=1)
    nc.vector.reduce_sum(out=ssm, in_=sex, axis=AX)
    src_ = tmp.tile([128, 2, ns, 1], F32, name=f"src__{g}", tag=f"src__{g}", bufs=1)
    nc.vector.reciprocal(src_, ssm)
    # cc = sex * srec * w   (srec, w broadcast over K via step-0 APs)
    def _b0(ap):
        a = list(ap.ap)
        a[-1] = [0, K]
        return bass.AP(tensor=ap.tensor, offset=ap.offset, ap=a)
    cca = blk.tile([128, 2, ns, K], F32, name=f"cca_{g}", tag=f"cca_{g}")
    nc.vector.tensor_tensor(out=cca, in0=sex, in1=_b0(src_[:, :, :, 0:1]), op=OP.mult)
    for lt in range(2):
        wcol = wT[lt].rearrange("p (s o) -> p s o", o=1)
        nc.vector.tensor_tensor(
            out=cca[:, lt, :, :], in0=cca[:, lt, :, :], in1=_b0(wcol), op=OP.mult
        )
    # alpha = cc * sqv1 * exp(-0.5*ln(cc^2*var1+eps))  (reuse sex as scratch)
    nc.vector.tensor_mul(sex, cca, cca)
    nc.vector.tensor_tensor(out=sex, in0=sex, in1=_b0(mv1[:, :, :, 1:2]), op=OP.mult)
    nc.scalar.activation(out=sex, in_=sex, func=AF.Ln, bias=c["epsc"], scale=1.0)
    nc.scalar.activation(out=sex, in_=sex, func=AF.Exp, bias=0.0, scale=-0.5)
    nc.vector.tensor_mul(cca, cca, sex)
    nc.vector.tensor_tensor(out=cca, in0=cca, in1=_b0(sqv1[:, :, :, 0:1]), op=OP.mult)
    alpha = cca  # [128, lt, st, K]

    if dbg is not None and g == 0:
        nc.sync.dma_start(out=dbg["lgTn"][:, :, :, :], in_=lgTn)
        nc.sync.dma_start(out=dbg["khTn"][:, :, :, :], in_=khTn)
        nc.sync.dma_start(out=dbg["rla"][:, :, :, :], in_=rla)
        nc.sync.dma_start(out=dbg["wT0"][:, :], in_=wT[0])
        nc.sync.dma_start(out=dbg["wT1"][:, :], in_=wT[1])
        nc.sync.dma_start(out=dbg["cca"][:, :, :, :], in_=alpha)
        nc.sync.dma_start(out=dbg["mv1"][:, :, :, :], in_=mv1)
        nc.sync.dma_start(out=dbg["lga"][:, :, :, :], in_=lga)

    # ---- phase 10: final combine into per-(b,lt) staging + 1 DMA each ----
    # Per-k engine schedule: p1[k] picks pass-1 (tA = aG*lgaG [+ b5x2]):
    #   'D' dve-STT (bias incl), 'T' dve-TS 4x mult (no bias -> batch-bias),
    #   'A' ACT mult + pool plain-TT bias add (pool CANNOT run STT: walrus
    #       rejects TensorScalarPtr on the Pool engine).
    # p2[k] is always DVE STT. 'T' ks must be contiguous (one batch-bias TT).
    p1, p2 = fmode
    for bi in range(nb):
        b = seqs[bi]
        stL, stG = bi, nb + bi
        for lt in range(2):
            n = LT[lt]
            ot = op.tile([128, K, D], BF16, name=f"ot_{b}_{lt}", tag="ot")
            need_bias = False
            for k in range(K):
                tA = op.tile([128, D], BF16, name=f"tA_{b}_{k}_{lt}", tag="tA")
                aG = alpha[:n, lt, stG, k : k + 1]
                aL = alpha[:n, lt, stL, k : k + 1]
                v1 = p1[k]
                if v1 == 'D':
                    nc.vector.scalar_tensor_tensor(
                        out=tA[:n, :], in0=lga[:n, lt, stG, :], scalar=aG,
                        in1=c["b5x2bc"][:n, :], op0=OP.mult, op1=OP.add)
                elif v1 == 'T':
                    nc.vector.tensor_scalar_mul(tA[:n, :], lga[:n, lt, stG, :], aG)
                    need_bias = True
                else:  # 'A': ACT mult, bias added on pool
                    nc.scalar.activation(
                        out=tA[:n, :], in_=lga[:n, lt, stG, :], func=AF.Identity,
                        bias=0.0, scale=aG)
                    nc.gpsimd.tensor_add(tA[:n, :], tA[:n, :], c["b5x2bc"][:n, :])
                nc.vector.scalar_tensor_tensor(
                    out=ot[:n, k, :], in0=lga[:n, lt, stL, :], scalar=aL,
                    in1=tA[:n, :], op0=OP.mult, op1=OP.add)
            if need_bias:
                nb_ks = [k for k in range(K) if p1[k] == 'T']
                k0, k1 = nb_ks[0], nb_ks[-1] + 1
                assert nb_ks == list(range(k0, k1)), "T ks must be contiguous"
                b5rep = bass.AP(
                    tensor=c["b5x2bc"].tensor, offset=c["b5x2bc"].offset,
                    ap=[[c["b5x2bc"].ap[0][0], n], [0, k1 - k0], [1, D]])
                nc.vector.tensor_tensor(
                    out=ot[:n, k0:k1, :], in0=ot[:n, k0:k1, :], in1=b5rep, op=OP.add)
            dst = out_t[b, :, lt * 128 : lt * 128 + n, :]
            qeng = nc.scalar if lmode == 'actq' else nc.sync
            qeng.dma_start(out=dst.rearrange("k l d -> l k d"), in_=ot[:n, :, :])


def build_module(reps=1, timing=False, fmode=('AAAAAATT', 'DDDDDDDD'), amode='pool', pbufs=4, kmix=2, vmode='dve', groups=2, obufs=6, s6b=4, lmode='dve', tmpb=2, dbg=False):
    """timing=True swaps the big I/O tensors to Internal DRAM so timing runs
    move ~0 bytes over the axon tunnel (values become garbage; latency
    behavior is identical since all engine ops are value-independent)."""
    nc = bacc.Bacc("TRN2", target_bir_lowering=False, debug=False, num_devices=NCORES)
    big = "Internal" if timing else "ExternalInput"
    t = {}
    t["xL"] = nc.dram_tensor("xL", [BPC, L, D], BF16, kind=big)
    t["xG"] = nc.dram_tensor("xG", [BPC, L, D], BF16, kind=big)
    t["slf"] = nc.dram_tensor("slf", [BPC, 1], F32, kind="ExternalInput")
    t["pos"] = nc.dram_tensor("pos", [L, D], BF16, kind="ExternalInput")
    t["rou"] = nc.dram_tensor("rou", [1, D], F32, kind="ExternalInput")
    t["wwt"] = nc.dram_tensor("wwt", [D, D], BF16, kind="ExternalInput")
    t["wb"] = nc.dram_tensor("wb", [D, 1], F32, kind="ExternalInput")
    t["g3"] = nc.dram_tensor("g3", [1, D], F32, kind="ExternalInput")
    t["b3"] = nc.dram_tensor("b3", [1, D], F32, kind="ExternalInput")
    t["g4"] = nc.dram_tensor("g4", [D, 1], F32, kind="ExternalInput")
    t["g4r"] = nc.dram_tensor("g4r", [1, D], F32, kind="ExternalInput")
    for _nm in ("rouc", "g3g4", "b3g4", "g3c", "b3c"):
        t[_nm] = nc.dram_tensor(_nm, [D, 1], F32, kind="ExternalInput")
    t["b4"] = nc.dram_tensor("b4", [D, 1], F32, kind="ExternalInput")
    t["g5"] = nc.dram_tensor("g5", [1, D], BF16, kind="ExternalInput")
    t["b5x2"] = nc.dram_tensor("b5x2", [1, D], BF16, kind="ExternalInput")
    t["m2t"] = nc.dram_tensor("m2t", [D, K], BF16, kind="ExternalInput")
    t["c2"] = nc.dram_tensor("c2", [1, K], F32, kind="ExternalInput")
    t["eye"] = nc.dram_tensor("eye", [128, 128], BF16, kind="ExternalInput")
    t["eye32"] = nc.dram_tensor("eye32", [NS, NS], F32, kind="ExternalInput")
    t["iop1"] = nc.dram_tensor("iop1", [D, 1], F32, kind="ExternalInput")
    out_t = nc.dram_tensor("out", [BPC, K, L, D], BF16,
                           kind="Internal" if timing else "ExternalOutput")
    sink_t = None
    if timing:
        sink_t = nc.dram_tensor("sink", [1, 4], F32, kind="ExternalOutput")
    dbgt = None
    if dbg:
        ns0 = 2 * (BPC // groups)
        dbgt = {
            "lgTn": nc.dram_tensor("d_lgTn", [128, ns0, 2, 256], BF16, kind="ExternalOutput"),
            "khTn": nc.dram_tensor("d_khTn", [128, ns0, 2, 256], BF16, kind="ExternalOutput"),
            "rla": nc.dram_tensor("d_rla", [128, ns0, 2, 200], BF16, kind="ExternalOutput"),
            "wT0": nc.dram_tensor("d_wT0", [128, ns0], F32, kind="ExternalOutput"),
            "wT1": nc.dram_tensor("d_wT1", [128, ns0], F32, kind="ExternalOutput"),
            "cca": nc.dram_tensor("d_cca", [128, 2, ns0, K], F32, kind="ExternalOutput"),
            "qpre": nc.dram_tensor("d_qpre", [ns0, D], F32, kind="ExternalOutput"),
            "mv1": nc.dram_tensor("d_mv1", [128, 2, ns0, 2], F32, kind="ExternalOutput"),
            "lga": nc.dram_tensor("d_lga", [128, 2, ns0, D], BF16, kind="ExternalOutput"),
        }

    with tile.TileContext(nc) as tc:
        with ExitStack() as ctx:
            c = _emit_consts(nc, tc, ctx, t)
            blk = ctx.enter_context(tc.tile_pool(name="blk", bufs=1))
            tmp = ctx.enter_context(tc.tile_pool(name="tmp", bufs=tmpb))
            ps = ctx.enter_context(tc.tile_pool(name="ps", bufs=pbufs, space="PSUM"))
            psR = ctx.enter_context(tc.tile_pool(name="psR", bufs=1, space="PSUM"))
            psq = ctx.enter_context(tc.tile_pool(name="psq", bufs=1, space="PSUM"))
            op = ctx.enter_context(tc.tile_pool(name="outp", bufs=obufs))
            pools = (blk, tmp, ps, psR, psq, op)
            if reps == 1:
                _emit_body(nc, tc, c, pools, t, out_t, fmode, amode, kmix, vmode, groups, s6b, lmode, dbgt)
            else:
                with tc.For_i(0, reps, 1):
                    _emit_body(nc, tc, c, pools, t, out_t, fmode, amode, kmix, vmode, groups, s6b, lmode)
            if sink_t is not None:
                snk = tmp.tile([1, 4], BF16, name="snk", tag="snk", bufs=1)
                nc.sync.dma_start(out=snk, in_=out_t[0, 0, 0:1, 0:4])
                nc.gpsimd.dma_start(out=sink_t[:, :], in_=snk)
    nc.compile()
    return nc


def host_inputs(local_item_emb, global_item_emb, intentions, pos_fai, rou, W_w, W_b,
                g1, b1, g2, b2, g3, b3, g4, b4, g5, b5, seq_len):
    """Host-side param folding + per-core sharding. Returns in_maps list."""
    f = np.float32
    xL = np.asarray(local_item_emb, f).astype(NPBF)
    xG = np.asarray(global_item_emb, f).astype(NPBF)
    g1, b1, g2, b2 = (np.asarray(v, f) for v in (g1, b1, g2, b2))
    g3, b3, g4, b4 = (np.asarray(v, f) for v in (g3, b3, g4, b4))
    g5, b5 = np.asarray(g5, f), np.asarray(b5, f)
    intentions = np.asarray(intentions, f)
    mu = intentions.mean(-1, keepdims=True)
    var = ((intentions - mu) ** 2).mean(-1, keepdims=True)
    ln2 = (intentions - mu) / np.sqrt(var + EPS) * g2 + b2          # [K, D]
    assert np.abs(g5).min() > 1e-3, "g5 too small for M2 folding"
    m2t = np.ascontiguousarray((ln2 * (g1 / g5)[None, :]).T, f)     # [D, K]
    c2 = (ln2 @ b1.astype(np.float64)).astype(f).reshape(1, K)      # [1, K]
    shared = {
        "pos": np.ascontiguousarray(pos_fai, f).astype(NPBF),
        "rou": np.asarray(rou, f).reshape(1, D),
        "wwt": np.ascontiguousarray((np.asarray(W_w, f) * g4[None, :]).T).astype(NPBF),
        "wb": (np.asarray(W_w, f) @ b4 + np.asarray(W_b, f)).reshape(D, 1),
        "g3": g3.reshape(1, D), "b3": b3.reshape(1, D),
        "g4": g4.reshape(D, 1), "b4": b4.reshape(D, 1), "g4r": g4.reshape(1, D),
        "rouc": np.asarray(rou, f).reshape(D, 1),
        "g3g4": (g3 * g4).reshape(D, 1), "b3g4": (b3 * g4).reshape(D, 1),
        "g3c": g3.reshape(D, 1), "b3c": b3.reshape(D, 1),
        "g5": g5.reshape(1, D).astype(NPBF),
        "b5x2": (2.0 * b5).reshape(1, D).astype(NPBF),
        "m2t": m2t.astype(NPBF), "c2": c2,
        "eye": np.eye(128, dtype=f).astype(NPBF),
        "eye32": np.eye(NS, dtype=f),
        "iop1": (np.arange(1, D + 1, dtype=f)).reshape(D, 1),
    }
    slf = np.asarray(seq_len).astype(f).reshape(B, 1)
    in_maps = []
    for cix in range(NCORES):
        s = slice(cix * BPC, (cix + 1) * BPC)
        in_maps.append(
            {"xL": np.ascontiguousarray(xL[s]), "xG": np.ascontiguousarray(xG[s]),
             "slf": np.ascontiguousarray(slf[s]), **shared}
        )
    return in_maps


_module_cache = {}


def kernel(**inputs) -> np.ndarray:
    in_maps = host_inputs(**inputs)
    if 1 not in _module_cache:
        _module_cache[1] = build_module(reps=1)
    nc = _module_cache[1]
    r = run_bass_kernel_spmd(nc, in_maps, list(range(NCORES)))
    out = np.concatenate([r.results[cix]["out"] for cix in range(NCORES)], axis=0)
    return out.astype(np.float32)
